# revision 19
# baseline (speedup 1.0000x reference)
"""Trainium2 Bass kernel for nn_CostSensitiveCrossEntropyLossN.

Reference semantics (B=131072 samples, C=1000 classes):
    log_probs = log_softmax(outputs)            # [B, C]
    predicted = argmax(outputs, axis=1)         # [B]
    cm = cost_matrix; cm[t_i, p_i] += 1 per sample
    cm = cm * (1 - eye) + 1;  mn = min(cm); mx = max(cm)
    cm = 1 + (cm - mn) / (mx - mn)
    loss = -mean_i(log_probs[i, t_i]) * mean_i(cm[t_i, p_i])

Key identities used:
    mean_i cm[t_i, p_i] is computable from the (t, p) count matrix:
        sum_i cm_masked[t_i, p_i] = sum_{a,b} counts[a,b] * cm_masked[a,b]
    so no per-sample gather of the normalized matrix is needed.

Distribution (8 NeuronCores, data-parallel over batch):
  Host assigns samples round-robin to cores, then sorts each core's shard by
  target class into 8 aligned 128-class "windows" (classes padded to 1024).
  Each window's sample count is padded to a uniform tile count across cores
  so the compiled program (one SPMD program) has a static, core-independent
  schedule. Pad samples carry an all-zero target one-hot (excluded from
  count/u matmuls) and valid=0 (excluded from the lse sum).

Numerics: x is bf16 on device. The host bf16-rounds x and then demotes every
non-first per-row maximum by one ulp, so the device winner one-hot
(x == rowmax) is exactly single-winner and matches first-occurrence argmax
on the rounded values (verified loss rel err ~4e-6 vs the f32 reference).

Per 128-sample tile on device (slow GpSimd does only the collective):
  ACT: exp(x) with fused row-sum accumulation (-> lse later via Ln)
  DVE: row max via bf16 pair-max + half-width reduce (fused over 2 tiles);
       winner one-hot wp = (x == rowmax) -> bf16
  PE:  counts_psum[w] += onehot_t^T @ wp      (bf16; exact integer counts)
       u_psum[w]      += onehot_t^T @ x[:, window]  (diag -> sum x[i,t_i])
The target one-hots are host-built and streamed fused with x: one
[128, 8*(1000+128)] bf16 DMA per 8-tile batch (contiguous per partition).
Counts windows are staged to DRAM during the loop; one ReduceScatter at loop
end overlaps the lse phase. Each core then reduces its class block to 5
scalars (-mn, mx, S, usum, lsum) written to its output; the host unshards by
combining the 8 cores' partials into the final loss.
"""
import os
import numpy as np
import ml_dtypes

NCORE = 8
P = 128
C = 1000
W = C + P           # fused x+onehot row stride
NW = 8              # class windows (classes padded to NW*P = 1024)
B_TOT = 131072
BETA1, BETA2 = 1.0, 2.0
KB = 8              # tiles fetched per dma_start (~2.3 MiB batches)


# ----------------------------------------------------------------------------
# Host-side prep
# ----------------------------------------------------------------------------

def _host_prep(targets):
    t = np.asarray(targets).astype(np.int64)
    tw_all = t // P
    per_cw = [[None] * NW for _ in range(NCORE)]
    for w in range(NW):
        sel = np.where(tw_all == w)[0]
        sel = sel[np.argsort(t[sel], kind="stable")]
        # deal this window's samples round-robin across cores (balanced +-1)
        for c in range(NCORE):
            per_cw[c][w] = sel[c::NCORE]
    T_w = []
    for w in range(NW):
        n_max = max(len(per_cw[c][w]) for c in range(NCORE))
        T_w.append(max(1, -(-n_max // P)))
    T = int(sum(T_w))
    rows = np.zeros((NCORE, T * P), dtype=np.int64)
    tloc = np.full((NCORE, T * P), -1.0, dtype=np.float32)
    valid = np.zeros((NCORE, T * P), dtype=np.float32)
    win_of_tile = np.concatenate(
        [np.full(T_w[w], w, dtype=np.int64) for w in range(NW)])
    for c in range(NCORE):
        off = 0
        for w in range(NW):
            sel = per_cw[c][w]
            n = len(sel)
            cap = T_w[w] * P
            rows[c, off:off + n] = sel
            rows[c, off + n:off + cap] = sel[0] if n > 0 else 0
            tloc[c, off:off + n] = (t[sel] - P * w).astype(np.float32)
            valid[c, off:off + n] = 1.0
            off += cap
    return rows, tloc, valid, win_of_tile, T


def _demote_tied_maxima(xb):
    """bf16-round x, then push every non-first per-row maximum down by 1 ulp
    so the device winner one-hot (x == rowmax) is exactly single-winner and
    matches first-occurrence argmax. Row maxima of randn rows are positive,
    so a uint16 decrement is the next-lower bf16."""
    xf = xb.astype(np.float32)
    wp = xf == xf.max(axis=1, keepdims=True)
    first = wp.argmax(axis=1)
    wp[np.arange(xb.shape[0]), first] = False
    r, cidx = np.nonzero(wp)
    xb.view(np.uint16)[r, cidx] -= 1
    return xb


def _build_inputs(outputs, targets, cost_matrix):
    rows, tloc, valid, win_of_tile, T = _host_prep(targets)
    xb = np.ascontiguousarray(
        np.asarray(outputs, dtype=np.float32)).astype(ml_dtypes.bfloat16)
    xb = _demote_tied_maxima(xb)
    cost_pad = np.zeros((NW * P, C), dtype=np.float32)
    cost_pad[:C] = np.asarray(cost_matrix, dtype=np.float32)
    ident = np.eye(P, dtype=np.float32)
    in_maps = []
    for c in range(NCORE):
        # fused per-partition stream: aug[p, j*W + 0:C] = x row of sample
        # (j, p); aug[p, j*W + C:W] = its target one-hot (class-in-window)
        xr = xb[rows[c]].reshape(T, P, C).transpose(1, 0, 2)   # [P, T, C]
        tl = tloc[c].reshape(T, P)
        O = np.zeros((T, P, P), dtype=ml_dtypes.bfloat16)
        jj, pp = np.nonzero(tl >= 0)
        O[jj, pp, tl[jj, pp].astype(np.int64)] = 1.0
        aug = np.concatenate([xr, O.transpose(1, 0, 2)], axis=2)
        aug = np.ascontiguousarray(aug.reshape(P, T * W))
        eyec = np.zeros((P, C), dtype=np.float32)
        for r in range(P):
            g = c * P + r
            if g < C:
                eyec[r, g] = 1.0
        eyem = 1.0 - eyec
        cost_c = cost_pad[c * P:(c + 1) * P]
        # cm2 = (counts + cost + 1)*eyem + eyec  ==  counts*eyem + bc
        bc = (cost_c + 1.0) * eyem + eyec
        in_maps.append({
            "aug": aug,
            "valid": np.ascontiguousarray(valid[c].reshape(T, P).T),
            "ident": ident,
            "eyem": np.ascontiguousarray(eyem),
            "bc": np.ascontiguousarray(bc),
            "mA": np.array([1, 1, 0, 0, 0, 0, 0, 0], dtype=np.float32)[:, None],
            "mB": np.array([0, 0, 1, 1, 1, 0, 0, 0], dtype=np.float32)[:, None],
        })
    return in_maps, win_of_tile, T


# ----------------------------------------------------------------------------
# Device program
# ----------------------------------------------------------------------------

def _build_program(T, win_of_tile):
    import concourse.bacc as bacc
    import concourse.tile as tile
    import concourse.mybir as mybir

    f32 = mybir.dt.float32
    bf16 = mybir.dt.bfloat16
    i8 = mybir.dt.int8
    ALU = mybir.AluOpType
    AF = mybir.ActivationFunctionType

    nc = bacc.Bacc("TRN2", target_bir_lowering=False, debug=False,
                   num_devices=NCORE)

    aug_d = nc.dram_tensor("aug", [P, T * W], bf16, kind="ExternalInput").ap()
    valid_d = nc.dram_tensor("valid", [P, T], f32, kind="ExternalInput").ap()
    eyem_d = nc.dram_tensor("eyem", [P, C], f32, kind="ExternalInput").ap()
    bc_d = nc.dram_tensor("bc", [P, C], f32, kind="ExternalInput").ap()
    ident_d = nc.dram_tensor("ident", [P, P], f32, kind="ExternalInput").ap()
    mA_d = nc.dram_tensor("mA", [8, 1], f32, kind="ExternalInput").ap()
    mB_d = nc.dram_tensor("mB", [8, 1], f32, kind="ExternalInput").ap()
    out_d = nc.dram_tensor("out", [8, 1], f32, kind="ExternalOutput").ap()

    first = np.zeros(T, dtype=bool)
    last = np.zeros(T, dtype=bool)
    for j in range(T):
        w = win_of_tile[j]
        first[j] = (j == 0) or (win_of_tile[j - 1] != w)
        last[j] = (j == T - 1) or (win_of_tile[j + 1] != w)

    replica = [list(range(NCORE))]

    with tile.TileContext(nc) as tc:
        with (
            tc.tile_pool(name="io", bufs=1) as io,
            tc.tile_pool(name="xs", bufs=3) as xs,
            tc.tile_pool(name="work", bufs=3) as work,
            tc.tile_pool(name="cw", bufs=2) as cwp,
            tc.tile_pool(name="accum", bufs=1) as acc,
            tc.tile_pool(name="ph2", bufs=1) as ph2,
            tc.tile_pool(name="psA", bufs=2, space="PSUM") as psA,
            tc.tile_pool(name="psB", bufs=2, space="PSUM") as psB,
            tc.tile_pool(name="psU", bufs=2, space="PSUM") as psU,
            tc.tile_pool(name="psT", bufs=1, space="PSUM") as psT,
            tc.tile_pool(name="dram", bufs=1, space="DRAM") as dram,
        ):
            # persistent inputs
            valid_sb = io.tile([P, T], f32)
            eyem_sb = io.tile([P, C], f32)
            bc_sb = io.tile([P, C], f32)
            ident_sb = io.tile([P, P], f32)
            mA_sb = io.tile([8, 1], f32)
            mB_sb = io.tile([8, 1], f32)
            for sb, d in ((valid_sb, valid_d), (eyem_sb, eyem_d),
                          (bc_sb, bc_d), (ident_sb, ident_d),
                          (mA_sb, mA_d), (mB_sb, mB_d)):
                nc.sync.dma_start(out=sb[:], in_=d)

            # persistent accumulators
            s_sb = acc.tile([P, T], f32)          # row sum(exp)
            lse_sb = acc.tile([P, T], f32)
            u_sb = acc.tile([P, NW, P], f32)
            udiag_sb = acc.tile([P, NW], f32)
            nc.vector.memset(u_sb[:], 0.0)

            counts_dram = dram.tile([NW * P, C], i8)
            counts_rs = dram.tile([P, C], i8)

            cA = cB = uP = None
            augt = None
            m2 = None
            for j in range(T):
                w = int(win_of_tile[j])
                wlo = w * P
                whi = min(C, wlo + P)
                ncls = whi - wlo

                # one contiguous ~2.3 MiB DMA per KB tiles (x + one-hots)
                if j % KB == 0:
                    kk = min(KB, T - j)
                    augt = xs.tile([P, KB * W], bf16, tag="aug")
                    nc.sync.dma_start(out=augt[:, 0:kk * W],
                                      in_=aug_d[:, j * W:(j + kk) * W])
                jj = j % KB
                xt = augt[:, jj * W:jj * W + C]
                oh_b = augt[:, jj * W + C:(jj + 1) * W]

                # ACT: exp + row-sum
                e_scr = work.tile([P, C], bf16, tag="e")
                nc.scalar.activation(out=e_scr[:], in_=xt, func=AF.Exp,
                                     accum_out=s_sb[:, j:j + 1])

                # DVE: row max fused over a tile pair — bf16 pair-max then
                # half-width reduce; per-tile winner one-hot
                if j % 2 == 0:
                    kk2 = min(2, T - j)
                    pair = augt[:, jj * W:(jj + kk2) * W].rearrange(
                        "p (k w) -> p k w", k=kk2)
                    h2 = work.tile([P, 2, C // 2], bf16, tag="h")
                    nc.vector.tensor_tensor(out=h2[:, 0:kk2, :],
                                            in0=pair[:, :, 0:C // 2],
                                            in1=pair[:, :, C // 2:C],
                                            op=ALU.max)
                    h3 = work.tile([P, 2, C // 4], bf16, tag="h3")
                    nc.vector.tensor_tensor(out=h3[:, 0:kk2, :],
                                            in0=h2[:, 0:kk2, 0:C // 4],
                                            in1=h2[:, 0:kk2, C // 4:C // 2],
                                            op=ALU.max)
                    m2 = work.tile([P, 2], f32, tag="m")
                    nc.vector.reduce_max(out=m2[:, 0:kk2], in_=h3[:, 0:kk2, :],
                                         axis=mybir.AxisListType.X)
                wp = work.tile([P, C], bf16, tag="wp")
                nc.vector.tensor_scalar(out=wp[:], in0=xt,
                                        scalar1=m2[:, j % 2:j % 2 + 1],
                                        scalar2=None, op0=ALU.is_equal)

                # PE: histogram + target-logit accumulation (all bf16)
                if first[j]:
                    cA = psA.tile([P, 500], f32, tag="cA")
                    cB = psB.tile([P, 500], f32, tag="cB")
                    uP = psU.tile([P, P], f32, tag="uP")
                nc.tensor.matmul(out=cA[:], lhsT=oh_b, rhs=wp[:, 0:500],
                                 start=bool(first[j]), stop=bool(last[j]))
                nc.tensor.matmul(out=cB[:], lhsT=oh_b, rhs=wp[:, 500:1000],
                                 start=bool(first[j]), stop=bool(last[j]))
                nc.tensor.matmul(out=uP[:, 0:ncls], lhsT=oh_b,
                                 rhs=xt[:, wlo:whi],
                                 start=bool(first[j]), stop=bool(last[j]))

                if last[j]:
                    # drain this window's counts to DRAM during the loop so
                    # the single ReduceScatter can start right at loop end
                    cw_sb = cwp.tile([P, C], i8, tag="cw")
                    nc.scalar.copy(out=cw_sb[:, 0:500], in_=cA[:])
                    nc.scalar.copy(out=cw_sb[:, 500:1000], in_=cB[:])
                    nc.scalar.copy(out=u_sb[:, w, 0:ncls], in_=uP[:, 0:ncls])
                    nc.sync.dma_start(out=counts_dram[w * P:(w + 1) * P, :],
                                      in_=cw_sb[:])

            # counts collective (GpSimd) overlaps the lse/udiag phase below
            nc.gpsimd.collective_compute(
                "ReduceScatter", ALU.add, replica_groups=replica,
                ins=[counts_dram[:].opt()], outs=[counts_rs[:].opt()])

            # lse = Ln(sum exp); masked sum over valid samples
            nc.scalar.activation(out=lse_sb[:], in_=s_sb[:], func=AF.Ln)
            lsum = ph2.tile([P, 1], f32)
            lse_junk = ph2.tile([P, T], f32)
            nc.vector.scalar_tensor_tensor(
                out=lse_junk[:], in0=lse_sb[:], scalar=1.0,
                in1=valid_sb[:], op0=ALU.mult, op1=ALU.mult,
                accum_out=lsum[:])

            # u diagonal per window -> sum (mask with identity, row-sum)
            diag_junk = ph2.tile([P, P], f32)
            for w in range(NW):
                nc.vector.scalar_tensor_tensor(
                    out=diag_junk[:], in0=u_sb[:, w, :], scalar=1.0,
                    in1=ident_sb[:], op0=ALU.mult, op1=ALU.mult,
                    accum_out=udiag_sb[:, w:w + 1])
            usum = ph2.tile([P, 1], f32)
            nc.vector.reduce_sum(out=usum[:], in_=udiag_sb[:],
                                 axis=mybir.AxisListType.X)

            # each core's ReduceScatter slice = its 128-class block
            crs_sb = ph2.tile([P, C], i8)
            nc.sync.dma_start(out=crs_sb[:], in_=counts_rs[:])
            crs32 = ph2.tile([P, C], f32)
            nc.scalar.copy(out=crs32[:], in_=crs_sb[:])

            # cm2 = (counts + cost + 1)*eyem + eyec == counts*eyem + bc
            cm = ph2.tile([P, C], f32)
            cm2 = ph2.tile([P, C], f32)
            nc.vector.tensor_tensor(out=cm2[:], in0=crs32[:], in1=eyem_sb[:],
                                    op=ALU.mult)
            nc.vector.tensor_tensor(out=cm2[:], in0=cm2[:], in1=bc_sb[:],
                                    op=ALU.add)

            # per-core partials: -mn (negated so max combines it), mx, S
            pvec = ph2.tile([P, 8], f32)
            nc.vector.memset(pvec[:], 0.0)
            nc.vector.tensor_reduce(out=pvec[:, 0:1], in_=cm2[:],
                                    axis=mybir.AxisListType.X, op=ALU.min,
                                    negate=True)
            nc.vector.tensor_reduce(out=pvec[:, 1:2], in_=cm2[:],
                                    axis=mybir.AxisListType.X, op=ALU.max)
            nc.vector.scalar_tensor_tensor(
                out=cm[:], in0=crs32[:], scalar=1.0, in1=cm2[:],
                op0=ALU.mult, op1=ALU.mult, accum_out=pvec[:, 2:3])
            nc.vector.tensor_copy(out=pvec[:, 3:4], in_=usum[:])
            nc.vector.tensor_copy(out=pvec[:, 4:5], in_=lsum[:])

            # transpose partials -> rows (partition k holds partial kind k);
            # combine across the 128 partitions: rows 0,1 via max, 2-4 via
            # sum (blend with 0/1 masks since engine APs start at partition 0)
            tp = psT.tile([8, P], f32)
            nc.tensor.transpose(out=tp[:], in_=pvec[:], identity=ident_sb[:])
            tv = ph2.tile([8, P], f32)
            nc.scalar.copy(out=tv[:], in_=tp[:])
            rmax = ph2.tile([8, 1], f32)
            radd = ph2.tile([8, 1], f32)
            scal_col = ph2.tile([8, 1], f32)
            nc.vector.tensor_reduce(out=rmax[:], in_=tv[:],
                                    axis=mybir.AxisListType.X, op=ALU.max)
            nc.vector.tensor_reduce(out=radd[:], in_=tv[:],
                                    axis=mybir.AxisListType.X, op=ALU.add)
            nc.vector.tensor_tensor(out=rmax[:], in0=rmax[:], in1=mA_sb[:],
                                    op=ALU.mult)
            nc.vector.tensor_tensor(out=radd[:], in0=radd[:], in1=mB_sb[:],
                                    op=ALU.mult)
            nc.vector.tensor_tensor(out=scal_col[:], in0=rmax[:], in1=radd[:],
                                    op=ALU.add)
            nc.sync.dma_start(out=out_d, in_=scal_col[:])

    nc.compile()
    return nc


# ----------------------------------------------------------------------------
# Entry points
# ----------------------------------------------------------------------------

def _prepare(outputs, targets, cost_matrix):
    in_maps, win_of_tile, T = _build_inputs(outputs, targets, cost_matrix)
    nc = _build_program(T, win_of_tile)
    return nc, in_maps


def _combine_partials(parts):
    """Host-side unshard: combine the 8 cores' 5 partials into the loss."""
    parts = np.asarray(parts, dtype=np.float32)        # [NCORE, 8]
    mn = np.float32(-parts[:, 0].max())
    mx = np.float32(parts[:, 1].max())
    S = np.float32(parts[:, 2].sum())
    U = np.float32(parts[:, 3].sum())
    L = np.float32(parts[:, 4].sum())
    glp = np.float32((U - L) / np.float32(B_TOT))
    gc = np.float32(BETA1 + (S / np.float32(B_TOT) - mn)
                    * np.float32(BETA2 - BETA1) / (mx - mn))
    return np.float32(-(glp * gc))


def _install_ntff_hook():
    """Register the axon NTFF profiling hook that the agent image's antenv
    stub lacks (mirrors trn_agent_boot's _ntff_profile_via_ctypes)."""
    import sys
    import types
    import ctypes
    import contextlib
    try:
        from antenv.axon_hooks import get_axon_ntff_profile_hook  # noqa
        return True
    except ImportError:
        pass
    so_path = "/opt/axon/libaxon_pjrt.so"
    if not os.path.exists(so_path):
        return False
    lib = ctypes.CDLL(so_path)
    if not hasattr(lib, "axon_start_nrt_profile"):
        return False
    lib.axon_start_nrt_profile.argtypes = [ctypes.POINTER(ctypes.c_int64),
                                           ctypes.c_size_t]
    lib.axon_start_nrt_profile.restype = ctypes.c_int64
    lib.axon_stop_nrt_profile.argtypes = [ctypes.c_char_p]
    lib.axon_stop_nrt_profile.restype = ctypes.c_int64

    @contextlib.contextmanager
    def _hook(output_dir, device_ids):
        import jax
        jax.devices()
        if device_ids:
            ids = (ctypes.c_int64 * len(device_ids))(*device_ids)
            rc = lib.axon_start_nrt_profile(ids, len(device_ids))
        else:
            rc = lib.axon_start_nrt_profile(None, 0)
        if rc != 0:
            raise RuntimeError(f"axon_start_nrt_profile rc={rc}")
        try:
            yield
        finally:
            n = lib.axon_stop_nrt_profile(str(output_dir).encode())
            print(f"ntff profile: {n} file(s) -> {output_dir}")

    mod = types.ModuleType("antenv.axon_hooks")
    mod.get_axon_ntff_profile_hook = lambda: _hook
    mod.set_axon_ntff_profile_hook = lambda h: None
    sys.modules["antenv.axon_hooks"] = mod
    return True


def kernel(outputs, targets, cost_matrix):
    targets = np.asarray(targets)
    nc, in_maps = _prepare(outputs, targets, cost_matrix)
    from concourse.bass_utils import run_bass_kernel_spmd
    trace = os.environ.get("KERNEL_TRACE", "0") == "1"
    if trace:
        trace = _install_ntff_hook()
    res = run_bass_kernel_spmd(nc, in_maps, list(range(NCORE)), trace=trace,
                               tmpdir=os.environ.get("KERNEL_TRACE_DIR"))
    if trace and res.exec_time_ns is not None:
        print(f"HW exec time: {res.exec_time_ns} ns")
    parts = [np.asarray(res.results[i]["out"]).reshape(8)
             for i in range(NCORE)]
    return _combine_partials(parts)


def kernel_sim(outputs, targets, cost_matrix):
    """CoreSim validation path (no hardware)."""
    import concourse.bass_interp as bass_interp
    nc, in_maps = _prepare(outputs, targets, cost_matrix)
    sim = bass_interp.MultiCoreSim(nc, num_cores=NCORE)
    for i in range(NCORE):
        for k, v in in_maps[i].items():
            sim.cores[i].tensor(k)[:] = v
    sim.simulate(check_with_hw=False)
    parts = [np.asarray(sim.cores[i].mem_tensor("out")).reshape(8)
             for i in range(NCORE)]
    return _combine_partials(parts)


# revision 22
# speedup vs baseline: 1.0822x; 1.0822x over previous
"""Trainium2 Bass kernel for nn_CostSensitiveCrossEntropyLossN.

Reference semantics (B=131072 samples, C=1000 classes):
    log_probs = log_softmax(outputs)            # [B, C]
    predicted = argmax(outputs, axis=1)         # [B]
    cm = cost_matrix; cm[t_i, p_i] += 1 per sample
    cm = cm * (1 - eye) + 1;  mn = min(cm); mx = max(cm)
    cm = 1 + (cm - mn) / (mx - mn)
    loss = -mean_i(log_probs[i, t_i]) * mean_i(cm[t_i, p_i])

Key identities used:
    mean_i cm[t_i, p_i] is computable from the (t, p) count matrix:
        sum_i cm_masked[t_i, p_i] = sum_{a,b} counts[a,b] * cm_masked[a,b]
    so no per-sample gather of the normalized matrix is needed.

Distribution (8 NeuronCores, data-parallel over batch):
  Host assigns samples round-robin to cores, then sorts each core's shard by
  target class into 8 aligned 128-class "windows" (classes padded to 1024).
  Each window's sample count is padded to a uniform tile count across cores
  so the compiled program (one SPMD program) has a static, core-independent
  schedule. Pad samples carry an all-zero target one-hot (excluded from
  count/u matmuls) and valid=0 (excluded from the lse sum).

Numerics: x is bf16 on device. The host bf16-rounds x and then demotes every
non-first per-row maximum by one ulp, so the device winner one-hot
(x == rowmax) is exactly single-winner and matches first-occurrence argmax
on the rounded values (verified loss rel err ~4e-6 vs the f32 reference).

Per 128-sample tile on device (slow GpSimd does only the collective):
  ACT: exp(x) with fused row-sum accumulation (-> lse later via Ln)
  DVE: row max via bf16 pair-max + half-width reduce (fused over 2 tiles);
       winner one-hot wp = (x == rowmax) -> bf16
  PE:  counts_psum[w] += onehot_t^T @ wp      (bf16; exact integer counts)
       u_psum[w]      += onehot_t^T @ x[:, window]  (diag -> sum x[i,t_i])
The target one-hots are host-built and streamed fused with x: one
[128, 8*(1000+128)] bf16 DMA per 8-tile batch (contiguous per partition).
Counts windows are staged to DRAM during the loop; one ReduceScatter at loop
end overlaps the lse phase. Each core then reduces its class block to 5
scalars (-mn, mx, S, usum, lsum) written to its output; the host unshards by
combining the 8 cores' partials into the final loss.
"""
import os
import numpy as np
import ml_dtypes

NCORE = 8
P = 128
C = 1000
W = C + P           # fused x+onehot row stride
NW = 8              # class windows (classes padded to NW*P = 1024)
B_TOT = 131072
BETA1, BETA2 = 1.0, 2.0
KB = 8              # tiles fetched per dma_start (~2.3 MiB batches)


# ----------------------------------------------------------------------------
# Host-side prep
# ----------------------------------------------------------------------------

def _host_prep(targets):
    t = np.asarray(targets).astype(np.int64)
    tw_all = t // P
    per_cw = [[None] * NW for _ in range(NCORE)]
    for w in range(NW):
        sel = np.where(tw_all == w)[0]
        sel = sel[np.argsort(t[sel], kind="stable")]
        # deal this window's samples round-robin across cores (balanced +-1)
        for c in range(NCORE):
            per_cw[c][w] = sel[c::NCORE]
    T_w = []
    for w in range(NW):
        n_max = max(len(per_cw[c][w]) for c in range(NCORE))
        T_w.append(max(1, -(-n_max // P)))
    T = int(sum(T_w))
    rows = np.zeros((NCORE, T * P), dtype=np.int64)
    tloc = np.full((NCORE, T * P), -1.0, dtype=np.float32)
    valid = np.zeros((NCORE, T * P), dtype=np.float32)
    win_of_tile = np.concatenate(
        [np.full(T_w[w], w, dtype=np.int64) for w in range(NW)])
    for c in range(NCORE):
        off = 0
        for w in range(NW):
            sel = per_cw[c][w]
            n = len(sel)
            cap = T_w[w] * P
            rows[c, off:off + n] = sel
            rows[c, off + n:off + cap] = sel[0] if n > 0 else 0
            tloc[c, off:off + n] = (t[sel] - P * w).astype(np.float32)
            valid[c, off:off + n] = 1.0
            off += cap
    return rows, tloc, valid, win_of_tile, T


def _demote_tied_maxima(xb):
    """bf16-round x, then push every non-first per-row maximum down by 1 ulp
    so the device winner one-hot (x == rowmax) is exactly single-winner and
    matches first-occurrence argmax. Row maxima of randn rows are positive,
    so a uint16 decrement is the next-lower bf16."""
    xf = xb.astype(np.float32)
    wp = xf == xf.max(axis=1, keepdims=True)
    first = wp.argmax(axis=1)
    wp[np.arange(xb.shape[0]), first] = False
    r, cidx = np.nonzero(wp)
    xb.view(np.uint16)[r, cidx] -= 1
    return xb


def _build_inputs(outputs, targets, cost_matrix):
    rows, tloc, valid, win_of_tile, T = _host_prep(targets)
    xb = np.ascontiguousarray(
        np.asarray(outputs, dtype=np.float32)).astype(ml_dtypes.bfloat16)
    xb = _demote_tied_maxima(xb)
    cost_pad = np.zeros((NW * P, C), dtype=np.float32)
    cost_pad[:C] = np.asarray(cost_matrix, dtype=np.float32)
    ident = np.eye(P, dtype=np.float32)
    in_maps = []
    for c in range(NCORE):
        # fused per-partition stream: aug[p, j*W + 0:C] = x row of sample
        # (j, p); aug[p, j*W + C:W] = its target one-hot (class-in-window)
        xr = xb[rows[c]].reshape(T, P, C).transpose(1, 0, 2)   # [P, T, C]
        tl = tloc[c].reshape(T, P)
        O = np.zeros((T, P, P), dtype=ml_dtypes.bfloat16)
        jj, pp = np.nonzero(tl >= 0)
        O[jj, pp, tl[jj, pp].astype(np.int64)] = 1.0
        aug = np.concatenate([xr, O.transpose(1, 0, 2)], axis=2)
        aug = np.ascontiguousarray(aug.reshape(P, T * W))
        eyec = np.zeros((P, C), dtype=np.float32)
        for r in range(P):
            g = c * P + r
            if g < C:
                eyec[r, g] = 1.0
        eyem = 1.0 - eyec
        cost_c = cost_pad[c * P:(c + 1) * P]
        # cm2 = (counts + cost + 1)*eyem + eyec  ==  counts*eyem + bc
        bc = (cost_c + 1.0) * eyem + eyec
        in_maps.append({
            "aug": aug,
            "valid": np.ascontiguousarray(valid[c].reshape(T, P).T),
            "ident": ident,
            "eyem": np.ascontiguousarray(eyem),
            "bc": np.ascontiguousarray(bc),
            "mA": np.array([1, 1, 0, 0, 0, 0, 0, 0], dtype=np.float32)[:, None],
            "mB": np.array([0, 0, 1, 1, 1, 0, 0, 0], dtype=np.float32)[:, None],
        })
    return in_maps, win_of_tile, T


# ----------------------------------------------------------------------------
# Device program
# ----------------------------------------------------------------------------

def _build_program(T, win_of_tile):
    import concourse.bacc as bacc
    import concourse.tile as tile
    import concourse.mybir as mybir

    f32 = mybir.dt.float32
    bf16 = mybir.dt.bfloat16
    ALU = mybir.AluOpType
    AF = mybir.ActivationFunctionType

    nc = bacc.Bacc("TRN2", target_bir_lowering=False, debug=False,
                   num_devices=NCORE)

    aug_d = nc.dram_tensor("aug", [P, T * W], bf16, kind="ExternalInput").ap()
    valid_d = nc.dram_tensor("valid", [P, T], f32, kind="ExternalInput").ap()
    eyem_d = nc.dram_tensor("eyem", [P, C], f32, kind="ExternalInput").ap()
    bc_d = nc.dram_tensor("bc", [P, C], f32, kind="ExternalInput").ap()
    ident_d = nc.dram_tensor("ident", [P, P], f32, kind="ExternalInput").ap()
    mA_d = nc.dram_tensor("mA", [8, 1], f32, kind="ExternalInput").ap()
    mB_d = nc.dram_tensor("mB", [8, 1], f32, kind="ExternalInput").ap()
    out_d = nc.dram_tensor("out", [8, 1], f32, kind="ExternalOutput").ap()

    first = np.zeros(T, dtype=bool)
    last = np.zeros(T, dtype=bool)
    for j in range(T):
        w = win_of_tile[j]
        first[j] = (j == 0) or (win_of_tile[j - 1] != w)
        last[j] = (j == T - 1) or (win_of_tile[j + 1] != w)

    replica = [list(range(NCORE))]

    with tile.TileContext(nc) as tc:
        with (
            tc.tile_pool(name="io", bufs=1) as io,
            tc.tile_pool(name="xs", bufs=3) as xs,
            tc.tile_pool(name="work", bufs=3) as work,
            tc.tile_pool(name="cw", bufs=2) as cwp,
            tc.tile_pool(name="accum", bufs=1) as acc,
            tc.tile_pool(name="ph2", bufs=1) as ph2,
            tc.tile_pool(name="psA", bufs=2, space="PSUM") as psA,
            tc.tile_pool(name="psB", bufs=2, space="PSUM") as psB,
            tc.tile_pool(name="psU", bufs=2, space="PSUM") as psU,
            tc.tile_pool(name="psT", bufs=1, space="PSUM") as psT,
            tc.tile_pool(name="dram", bufs=1, space="DRAM") as dram,
        ):
            # persistent inputs
            valid_sb = io.tile([P, T], f32)
            eyem_sb = io.tile([P, C], f32)
            bc_sb = io.tile([P, C], f32)
            ident_sb = io.tile([P, P], f32)
            mA_sb = io.tile([8, 1], f32)
            mB_sb = io.tile([8, 1], f32)
            for sb, d in ((valid_sb, valid_d), (eyem_sb, eyem_d),
                          (bc_sb, bc_d), (ident_sb, ident_d),
                          (mA_sb, mA_d), (mB_sb, mB_d)):
                nc.sync.dma_start(out=sb[:], in_=d)

            # persistent accumulators
            s_sb = acc.tile([P, T], f32)          # row sum(exp)
            lse_sb = acc.tile([P, T], f32)
            u_sb = acc.tile([P, NW, P], f32)
            udiag_sb = acc.tile([P, NW], f32)
            nc.vector.memset(u_sb[:], 0.0)

            counts_dram = dram.tile([NW * P, C], bf16)
            counts_rs = dram.tile([P, C], bf16)

            # batch schedule: short prologue batches so compute starts as
            # soon as the first tiles land, then steady ~2.3 MiB batches
            starts = [0, 2, 4, 8]
            while T - starts[-1] > KB:
                starts.append(starts[-1] + KB)
            batch_of = np.zeros(T, dtype=np.int64)
            batch_start = {}
            for bi, st in enumerate(starts):
                en = starts[bi + 1] if bi + 1 < len(starts) else T
                batch_of[st:en] = bi
                batch_start[bi] = st

            cA = cB = uP = None
            augt = None
            m2 = None
            for j in range(T):
                w = int(win_of_tile[j])
                wlo = w * P
                whi = min(C, wlo + P)
                ncls = whi - wlo

                bi = int(batch_of[j])
                bst = batch_start[bi]
                if j == bst:
                    ben = batch_start[bi + 1] if bi + 1 in batch_start else T
                    kk = ben - bst
                    augt = xs.tile([P, KB * W], bf16, tag="aug")
                    nc.sync.dma_start(out=augt[:, 0:kk * W],
                                      in_=aug_d[:, bst * W:(bst + kk) * W])
                jj = j - bst
                xt = augt[:, jj * W:jj * W + C]
                oh_b = augt[:, jj * W + C:(jj + 1) * W]

                # ACT: exp + row-sum
                e_scr = work.tile([P, C], bf16, tag="e")
                nc.scalar.activation(out=e_scr[:], in_=xt, func=AF.Exp,
                                     accum_out=s_sb[:, j:j + 1])

                # DVE: row max fused over a tile pair — bf16 pair-max then
                # half-width reduce; per-tile winner one-hot
                if j % 2 == 0:
                    kk2 = min(2, T - j)
                    pair = augt[:, jj * W:(jj + kk2) * W].rearrange(
                        "p (k w) -> p k w", k=kk2)
                    h2 = work.tile([P, 2, C // 2], bf16, tag="h")
                    nc.vector.tensor_tensor(out=h2[:, 0:kk2, :],
                                            in0=pair[:, :, 0:C // 2],
                                            in1=pair[:, :, C // 2:C],
                                            op=ALU.max)
                    m2 = work.tile([P, 2], f32, tag="m")
                    nc.vector.reduce_max(out=m2[:, 0:kk2], in_=h2[:, 0:kk2, :],
                                         axis=mybir.AxisListType.X)
                wp = work.tile([P, C], bf16, tag="wp")
                nc.vector.tensor_scalar(out=wp[:], in0=xt,
                                        scalar1=m2[:, j % 2:j % 2 + 1],
                                        scalar2=None, op0=ALU.is_equal)

                # PE: histogram + target-logit accumulation (all bf16)
                if first[j]:
                    cA = psA.tile([P, 500], f32, tag="cA")
                    cB = psB.tile([P, 500], f32, tag="cB")
                    uP = psU.tile([P, P], f32, tag="uP")
                nc.tensor.matmul(out=cA[:], lhsT=oh_b, rhs=wp[:, 0:500],
                                 start=bool(first[j]), stop=bool(last[j]))
                nc.tensor.matmul(out=cB[:], lhsT=oh_b, rhs=wp[:, 500:1000],
                                 start=bool(first[j]), stop=bool(last[j]))
                nc.tensor.matmul(out=uP[:, 0:ncls], lhsT=oh_b,
                                 rhs=xt[:, wlo:whi],
                                 start=bool(first[j]), stop=bool(last[j]))

                if last[j]:
                    # drain this window's counts to DRAM during the loop so
                    # the single ReduceScatter can start right at loop end
                    cw_sb = cwp.tile([P, C], bf16, tag="cw")
                    nc.scalar.copy(out=cw_sb[:, 0:500], in_=cA[:])
                    nc.scalar.copy(out=cw_sb[:, 500:1000], in_=cB[:])
                    nc.scalar.copy(out=u_sb[:, w, 0:ncls], in_=uP[:, 0:ncls])
                    nc.sync.dma_start(out=counts_dram[w * P:(w + 1) * P, :],
                                      in_=cw_sb[:])

            # counts collective (GpSimd) overlaps the lse/udiag phase below
            nc.gpsimd.collective_compute(
                "ReduceScatter", ALU.add, replica_groups=replica,
                ins=[counts_dram[:].opt()], outs=[counts_rs[:].opt()])

            # lse = Ln(sum exp); masked sum over valid samples
            nc.scalar.activation(out=lse_sb[:], in_=s_sb[:], func=AF.Ln)
            lsum = ph2.tile([P, 1], f32)
            lse_junk = ph2.tile([P, T], f32)
            nc.vector.scalar_tensor_tensor(
                out=lse_junk[:], in0=lse_sb[:], scalar=1.0,
                in1=valid_sb[:], op0=ALU.mult, op1=ALU.mult,
                accum_out=lsum[:])

            # u diagonal per window -> sum (mask with identity, row-sum)
            diag_junk = ph2.tile([P, P], f32)
            for w in range(NW):
                nc.vector.scalar_tensor_tensor(
                    out=diag_junk[:], in0=u_sb[:, w, :], scalar=1.0,
                    in1=ident_sb[:], op0=ALU.mult, op1=ALU.mult,
                    accum_out=udiag_sb[:, w:w + 1])
            usum = ph2.tile([P, 1], f32)
            nc.vector.reduce_sum(out=usum[:], in_=udiag_sb[:],
                                 axis=mybir.AxisListType.X)

            # each core's ReduceScatter slice = its 128-class block
            crs_sb = ph2.tile([P, C], bf16)
            nc.sync.dma_start(out=crs_sb[:], in_=counts_rs[:])
            crs32 = ph2.tile([P, C], f32)
            nc.scalar.copy(out=crs32[:], in_=crs_sb[:])

            # cm2 = (counts + cost + 1)*eyem + eyec == counts*eyem + bc
            cm = ph2.tile([P, C], f32)
            cm2 = ph2.tile([P, C], f32)
            nc.vector.tensor_tensor(out=cm2[:], in0=crs32[:], in1=eyem_sb[:],
                                    op=ALU.mult)
            nc.vector.tensor_tensor(out=cm2[:], in0=cm2[:], in1=bc_sb[:],
                                    op=ALU.add)

            # per-core partials: -mn (negated so max combines it), mx, S
            pvec = ph2.tile([P, 8], f32)
            nc.vector.memset(pvec[:], 0.0)
            nc.vector.tensor_reduce(out=pvec[:, 0:1], in_=cm2[:],
                                    axis=mybir.AxisListType.X, op=ALU.min,
                                    negate=True)
            nc.vector.tensor_reduce(out=pvec[:, 1:2], in_=cm2[:],
                                    axis=mybir.AxisListType.X, op=ALU.max)
            nc.vector.scalar_tensor_tensor(
                out=cm[:], in0=crs32[:], scalar=1.0, in1=cm2[:],
                op0=ALU.mult, op1=ALU.mult, accum_out=pvec[:, 2:3])
            nc.vector.tensor_copy(out=pvec[:, 3:4], in_=usum[:])
            nc.vector.tensor_copy(out=pvec[:, 4:5], in_=lsum[:])

            # transpose partials -> rows (partition k holds partial kind k);
            # combine across the 128 partitions: rows 0,1 via max, 2-4 via
            # sum (blend with 0/1 masks since engine APs start at partition 0)
            tp = psT.tile([8, P], f32)
            nc.tensor.transpose(out=tp[:], in_=pvec[:], identity=ident_sb[:])
            tv = ph2.tile([8, P], f32)
            nc.scalar.copy(out=tv[:], in_=tp[:])
            rmax = ph2.tile([8, 1], f32)
            radd = ph2.tile([8, 1], f32)
            scal_col = ph2.tile([8, 1], f32)
            nc.vector.tensor_reduce(out=rmax[:], in_=tv[:],
                                    axis=mybir.AxisListType.X, op=ALU.max)
            nc.vector.tensor_reduce(out=radd[:], in_=tv[:],
                                    axis=mybir.AxisListType.X, op=ALU.add)
            nc.vector.tensor_tensor(out=rmax[:], in0=rmax[:], in1=mA_sb[:],
                                    op=ALU.mult)
            nc.vector.tensor_tensor(out=radd[:], in0=radd[:], in1=mB_sb[:],
                                    op=ALU.mult)
            nc.vector.tensor_tensor(out=scal_col[:], in0=rmax[:], in1=radd[:],
                                    op=ALU.add)
            nc.sync.dma_start(out=out_d, in_=scal_col[:])

    nc.compile()
    return nc


# ----------------------------------------------------------------------------
# Entry points
# ----------------------------------------------------------------------------

def _prepare(outputs, targets, cost_matrix):
    in_maps, win_of_tile, T = _build_inputs(outputs, targets, cost_matrix)
    nc = _build_program(T, win_of_tile)
    return nc, in_maps


def _combine_partials(parts):
    """Host-side unshard: combine the 8 cores' 5 partials into the loss."""
    parts = np.asarray(parts, dtype=np.float32)        # [NCORE, 8]
    mn = np.float32(-parts[:, 0].max())
    mx = np.float32(parts[:, 1].max())
    S = np.float32(parts[:, 2].sum())
    U = np.float32(parts[:, 3].sum())
    L = np.float32(parts[:, 4].sum())
    glp = np.float32((U - L) / np.float32(B_TOT))
    gc = np.float32(BETA1 + (S / np.float32(B_TOT) - mn)
                    * np.float32(BETA2 - BETA1) / (mx - mn))
    return np.float32(-(glp * gc))


def _install_ntff_hook():
    """Register the axon NTFF profiling hook that the agent image's antenv
    stub lacks (mirrors trn_agent_boot's _ntff_profile_via_ctypes)."""
    import sys
    import types
    import ctypes
    import contextlib
    try:
        from antenv.axon_hooks import get_axon_ntff_profile_hook  # noqa
        return True
    except ImportError:
        pass
    so_path = "/opt/axon/libaxon_pjrt.so"
    if not os.path.exists(so_path):
        return False
    lib = ctypes.CDLL(so_path)
    if not hasattr(lib, "axon_start_nrt_profile"):
        return False
    lib.axon_start_nrt_profile.argtypes = [ctypes.POINTER(ctypes.c_int64),
                                           ctypes.c_size_t]
    lib.axon_start_nrt_profile.restype = ctypes.c_int64
    lib.axon_stop_nrt_profile.argtypes = [ctypes.c_char_p]
    lib.axon_stop_nrt_profile.restype = ctypes.c_int64

    @contextlib.contextmanager
    def _hook(output_dir, device_ids):
        import jax
        jax.devices()
        if device_ids:
            ids = (ctypes.c_int64 * len(device_ids))(*device_ids)
            rc = lib.axon_start_nrt_profile(ids, len(device_ids))
        else:
            rc = lib.axon_start_nrt_profile(None, 0)
        if rc != 0:
            raise RuntimeError(f"axon_start_nrt_profile rc={rc}")
        try:
            yield
        finally:
            n = lib.axon_stop_nrt_profile(str(output_dir).encode())
            print(f"ntff profile: {n} file(s) -> {output_dir}")

    mod = types.ModuleType("antenv.axon_hooks")
    mod.get_axon_ntff_profile_hook = lambda: _hook
    mod.set_axon_ntff_profile_hook = lambda h: None
    sys.modules["antenv.axon_hooks"] = mod
    return True


def kernel(outputs, targets, cost_matrix):
    targets = np.asarray(targets)
    nc, in_maps = _prepare(outputs, targets, cost_matrix)
    from concourse.bass_utils import run_bass_kernel_spmd
    trace = os.environ.get("KERNEL_TRACE", "0") == "1"
    if trace:
        trace = _install_ntff_hook()
    res = run_bass_kernel_spmd(nc, in_maps, list(range(NCORE)), trace=trace,
                               tmpdir=os.environ.get("KERNEL_TRACE_DIR"))
    if trace and res.exec_time_ns is not None:
        print(f"HW exec time: {res.exec_time_ns} ns")
    parts = [np.asarray(res.results[i]["out"]).reshape(8)
             for i in range(NCORE)]
    return _combine_partials(parts)


def kernel_sim(outputs, targets, cost_matrix):
    """CoreSim validation path (no hardware)."""
    import concourse.bass_interp as bass_interp
    nc, in_maps = _prepare(outputs, targets, cost_matrix)
    sim = bass_interp.MultiCoreSim(nc, num_cores=NCORE)
    for i in range(NCORE):
        for k, v in in_maps[i].items():
            sim.cores[i].tensor(k)[:] = v
    sim.simulate(check_with_hw=False)
    parts = [np.asarray(sim.cores[i].mem_tensor("out")).reshape(8)
             for i in range(NCORE)]
    return _combine_partials(parts)
